# revision 30
# baseline (speedup 1.0000x reference)
"""Incremental MADE autoregressive sampler on 8 TRN2 NeuronCores.

v2: ALL layer accumulators are persistent PSUM banks updated incrementally.

With hidden units degree-sorted, activations are append-only across the 32
autoregressive steps: once x_0..x_g are set, every hidden unit of degree <= g
is final. Exploit this at every layer:

- pre1 (L1 pre-activations): ping-pong PSUM banks per 128-block; one K=1
  rank-1 update per step (new coordinate), plus a single K=33 catch-up matmul
  from xT (with a ones-row folding in the bias) when a block first becomes
  the active cover.
- S2/S3 (L2/L3 pre-activations): ping-pong PSUM banks per cover block. Each
  step adds ONLY the newly finalized ~33-unit degree group (K~33 matmul).
  When a block is about to become cover, a catch-up chain (bias + full
  finalized chunks) accumulates the older prefix once. No per-step prefix
  recompute -> Tensor queue no longer head-of-line-blocks the serial chain.
- theta [64, B]: single persistent PSUM accumulator; per-step "touch" adds
  the new group's contribution to all output rows (K~33, M=64). The tail
  reads rows idx (mu) and 32+idx (log_std) straight out of PSUM.
- Tail: es=exp(theta_ls) (ACT) -> t2=es*z -> x_idx=t2+theta_mu written
  DIRECTLY into the xT row in SBUF (no DMA scatter; k1/catch-up read xT).

Batch: data-parallel over 8 cores (512 rows/core); each core runs two
256-wide column chains, phase-interleaved with a skew so the two serial
dependency chains overlap on different engines. Relus/tails are spread
across ACT/DVE/Pool(gpsimd) so no single elementwise engine saturates.
"""

import os
import sys
import numpy as np

for _p in ("/opt/trn_rl_repo", "/opt/pypackages"):
    if _p not in sys.path:
        sys.path.insert(0, _p)

import concourse.bass as bass
import concourse.tile as tile
from concourse import bacc
from concourse import mybir
from concourse.bass_utils import run_bass_kernel_spmd

D, H, L, B = 32, 1024, 2, 4096
NCORES = 8
BC = B // NCORES          # 512 batch rows per core
P = 128                   # partitions
NB = H // P               # 8 hidden blocks
F32 = mybir.dt.float32
MMDT = mybir.dt.bfloat16

STOP = int(os.environ.get("MADE_STOP", "32"))
NCH = int(os.environ.get("MADE_CHAINS", "2"))
SKEW = int(os.environ.get("MADE_SKEW", "7"))


def _schedule():
    """Static per-step schedule from the degree structure."""
    d_hid = np.arange(H) % (D - 1)
    perm = np.argsort(d_hid, kind="stable")
    ds = d_hid[perm]
    glo = [int(np.sum(ds < g)) for g in range(D - 1)]
    ghi = [int(np.sum(ds <= g)) for g in range(D - 1)]
    cov = {g: list(range(glo[g] // P, (ghi[g] - 1) // P + 1))
           for g in range(D - 1)}
    # src_slices(g): (block c, row lo, row hi) covering units [glo, ghi)
    srcs = {}
    for g in range(D - 1):
        sl = []
        for c in cov[g]:
            lo = max(glo[g], c * P) - c * P
            hi = min(ghi[g], (c + 1) * P) - c * P
            sl.append((c, lo, hi))
        srcs[g] = sl
    # first step idx at which block Bb is in cover
    first = {}
    for g in range(D - 1):
        for Bb in cov[g]:
            first.setdefault(Bb, g + 1)
    return perm, ds, glo, ghi, cov, srcs, first


def _host_prep(W0, b0, Wh, bh, Wout, bout):
    perm, ds, glo, ghi, cov, srcs, first = _schedule()
    d_in = np.arange(D)
    d_out = np.arange(D) - 1
    m0 = (ds[:, None] >= d_in[None, :]).astype(np.float32)        # [H, D]
    mh = (ds[:, None] >= ds[None, :]).astype(np.float32)          # [H, H]
    mo = (d_out[:, None] >= ds[None, :]).astype(np.float32)       # [D, H]
    mo = np.concatenate([mo, mo], axis=0)                         # [2D, H]

    W0p = m0 * W0[perm, :]                    # [H, D] (out, in)
    Wh0p = mh * Wh[0][perm][:, perm]          # [H, H] (out, in)
    Wh1p = mh * Wh[1][perm][:, perm]
    Wop = mo * Wout[:, perm]                  # [2D, H]

    w0b = np.concatenate([W0p.T, b0[perm][None, :]], axis=0)      # [D+1, H]
    wh0T = Wh0p.T.reshape(NB, P, H).copy()                        # [c][128, H]
    wh1T = Wh1p.T.reshape(NB, P, H).copy()

    # k1 packed columns: per (idx, Bb in cov(idx-1)): W0p[block Bb, idx-1]
    k1_off, k1_list = {}, []
    for idx in range(1, D):
        for Bb in cov[idx - 1]:
            k1_off[(idx, Bb)] = len(k1_list)
            k1_list.append(W0p[Bb * P:(Bb + 1) * P, idx - 1])
    w0k1 = np.concatenate(k1_list).reshape(1, -1)                 # [1, n*128]

    # group matmul lhsT slices, zero-masked outside group rows, base-0 K=128:
    # per (idx, Bb dst, c src): whT[c][:, Bb block] with rows outside
    # [lo, hi) zeroed -> [128, 128]
    def pack_grp(whT):
        off, lst = {}, []
        for idx in range(1, D):
            g = idx - 1
            for Bb in cov[g]:
                for (c, lo, hi) in srcs[g]:
                    blk = whT[c][:, Bb * P:(Bb + 1) * P].copy()
                    blk[:lo, :] = 0.0
                    blk[hi:, :] = 0.0
                    off[(idx, Bb, c)] = len(lst)
                    lst.append(blk)
        return off, np.concatenate(lst, axis=1)                   # [128, n*128]

    g2_off, wg2 = pack_grp(wh0T)
    g3_off, wg3 = pack_grp(wh1T)

    # touch lhsT slices: per (idx, c src): Wop[:, block c].T rows-masked
    to_off, to_list = {}, []
    for idx in range(1, D):
        g = idx - 1
        for (c, lo, hi) in srcs[g]:
            blk = Wop[:, c * P:(c + 1) * P].T.copy()              # [128, 2D]
            blk[:lo, :] = 0.0
            blk[hi:, :] = 0.0
            to_off[(idx, c)] = len(to_list)
            to_list.append(blk)
    wto = np.concatenate(to_list, axis=1)                         # [128, n*2D]

    # extract correction columns: per (idx, c): (ls, mu) col pair
    wpm = np.zeros((P, len(to_list) * 2), dtype=np.float32)
    for (idx, c), off in to_off.items():
        wpm[:, 2 * off] = to_list[off][:, D + idx]                # ls col
        wpm[:, 2 * off + 1] = to_list[off][:, idx]                # mu col

    # one-hot theta_sb row selectors: col idx = e_{D+idx}, col D+idx = e_idx
    ohx = np.zeros((2 * D, 2 * D), dtype=np.float32)
    for idx in range(D):
        ohx[D + idx, idx] = 1.0
        ohx[idx, D + idx] = 1.0

    return dict(w0b=w0b, wh0T=wh0T, wh1T=wh1T,
                w0k1=w0k1, wg2=wg2, wg3=wg3, wto=wto, wpm=wpm, ohx=ohx,
                k1_off=k1_off, g2_off=g2_off, g3_off=g3_off, to_off=to_off,
                n_k1=len(k1_list), n_g2=len(g2_off), n_g3=len(g3_off),
                n_to=len(to_list),
                bh0r=bh[0][perm][None, :], bh1r=bh[1][perm][None, :],
                boutr=bout[None, :],
                glo=glo, ghi=ghi, cov=cov, srcs=srcs, first=first)


def _build(prep):
    nc = bacc.Bacc("TRN2", target_bir_lowering=False, debug=False,
                   num_devices=NCORES)

    def din(name, shape, dt=MMDT):
        return nc.dram_tensor(name, list(shape), dt, kind="ExternalInput").ap()

    d_w0b = din("w0b", (D + 1, H))
    d_wh0 = din("wh0t", (NB, P, H))
    d_wh1 = din("wh1t", (NB, P, H))
    d_w0k1 = din("w0k1", (1, prep["n_k1"] * P))
    d_wg2 = din("wg2", (P, prep["n_g2"] * P))
    d_wg3 = din("wg3", (P, prep["n_g3"] * P))
    d_wto = din("wto", (P, prep["n_to"] * 2 * D))
    d_wpm = din("wpm", (P, prep["n_to"] * 2))
    d_ohx = din("ohx", (2 * D, 2 * D))
    d_bh0 = din("bh0r", (1, H))
    d_bh1 = din("bh1r", (1, H))
    d_bo = din("boutr", (1, 2 * D))
    d_z = din("zb", (D, BC), F32)
    d_out = nc.dram_tensor("out", [D, BC], F32, kind="ExternalOutput").ap()

    cov, srcs, first = prep["cov"], prep["srcs"], prep["first"]
    ghi = prep["ghi"]
    # catch-up for block Bb is emitted during step first[Bb]-1
    catch_at = {}
    for Bb, f in first.items():
        if Bb >= 1:
            catch_at.setdefault(f - 1, []).append(Bb)

    from contextlib import ExitStack
    with tile.TileContext(nc) as tc, ExitStack() as ctx:
        cp = ctx.enter_context(tc.tile_pool(name="const", bufs=1))
        pp = ctx.enter_context(tc.tile_pool(name="pers", bufs=1, space="PSUM"))

        w0b = cp.tile([D + 1, H], MMDT, tag="w0b")
        wh0 = [cp.tile([P, H], MMDT, tag=f"wh0_{c}", name=f"wh0_{c}")
               for c in range(NB)]
        wh1 = [cp.tile([P, H], MMDT, tag=f"wh1_{c}", name=f"wh1_{c}")
               for c in range(NB)]
        w0k1 = cp.tile([1, prep["n_k1"] * P], MMDT, tag="w0k1")
        wg2 = cp.tile([P, prep["n_g2"] * P], MMDT, tag="wg2")
        wg3 = cp.tile([P, prep["n_g3"] * P], MMDT, tag="wg3")
        wto = cp.tile([P, prep["n_to"] * 2 * D], MMDT, tag="wto")
        bh0r = cp.tile([1, H], MMDT, tag="bh0r")
        bh1r = cp.tile([1, H], MMDT, tag="bh1r")
        bor = cp.tile([1, 2 * D], MMDT, tag="bor")
        wpm = cp.tile([P, prep["n_to"] * 2], MMDT, tag="wpm")
        ohx = cp.tile([2 * D, 2 * D], MMDT, tag="ohx")
        theta_sb = cp.tile([2 * D, BC], MMDT, tag="theta_sb")
        ones = cp.tile([1, BC], MMDT, tag="ones")
        xT = cp.tile([D + 1, BC], MMDT, tag="xT")
        a1 = [cp.tile([P, BC], MMDT, tag=f"a1_{r}", name=f"a1_{r}") for r in range(NB)]
        a2 = [cp.tile([P, BC], MMDT, tag=f"a2_{r}", name=f"a2_{r}") for r in range(NB)]
        a3 = [cp.tile([P, BC], MMDT, tag=f"a3_{r}", name=f"a3_{r}") for r in range(NB)]
        es = cp.tile([1, BC], F32, tag="es")
        t2 = cp.tile([1, BC], F32, tag="t2")
        xi = [cp.tile([1, BC], MMDT, tag=f"xi{p}", name=f"xi{p}")
              for p in range(2)]
        outf = cp.tile([D, BC], F32, tag="outf")

        # persistent PSUM: pre1/S2/S3 ping-pong banks + theta accumulator +
        # extract tile (ls at row 0, mu at row 32: both %32-aligned)
        pre1 = [pp.tile([P, BC], F32, tag=f"pre1_{s}", name=f"pre1_{s}")
                for s in range(2)]
        s2 = [pp.tile([P, BC], F32, tag=f"s2_{s}", name=f"s2_{s}")
              for s in range(2)]
        s3 = [pp.tile([P, BC], F32, tag=f"s3_{s}", name=f"s3_{s}")
              for s in range(2)]
        theta = pp.tile([2 * D, BC], F32, tag="theta")
        ex = pp.tile([D + 1, BC], F32, tag="ex")

        # input DMAs, ordered by first use
        nc.sync.dma_start(bor[:], d_bo)
        nc.sync.dma_start(ohx[:], d_ohx)
        nc.sync.dma_start(w0b[:], d_w0b)
        nc.sync.dma_start(w0k1[:], d_w0k1)
        nc.sync.dma_start(wpm[:], d_wpm)
        nc.sync.dma_start(bh0r[:], d_bh0)
        nc.sync.dma_start(bh1r[:], d_bh1)
        nc.sync.dma_start(wg2[:], d_wg2)
        nc.sync.dma_start(wg3[:], d_wg3)
        nc.sync.dma_start(wto[:], d_wto)
        for c in range(NB):
            nc.sync.dma_start(wh0[c][:], d_wh0[c, :, :])
            nc.sync.dma_start(wh1[c][:], d_wh1[c, :, :])

        zrow = {}

        def fetch_z(i):
            if i < STOP and i not in zrow:
                zr_t = cp.tile([1, BC], F32, tag="zrow", bufs=4, name=f"zr{i}")
                zrow[i] = zr_t
                nc.sync.dma_start(zr_t[:], d_z[i:i + 1, :])

        for i in range(3):
            fetch_z(i)

        nc.vector.memset(xT[:], 0.0)
        nc.vector.memset(xT[D:D + 1, :], 1.0)
        nc.vector.memset(ones[:], 1.0)

        # theta := bout broadcast (rank-1); block-0 accumulators: bias+coords
        nc.tensor.matmul(theta[:, :], bor[0:1, :], ones[0:1, :],
                         start=True, stop=True, skip_group_check=True)
        nc.vector.tensor_scalar_add(theta_sb[:, :], theta[:, :], 0.0)
        nc.tensor.matmul(pre1[0], w0b[:, 0:P], xT[:, :],
                         start=True, stop=True, skip_group_check=True)
        nc.tensor.matmul(s2[0], bh0r[0:1, 0:P], ones[0:1, :],
                         start=True, stop=True, skip_group_check=True)
        nc.tensor.matmul(s3[0], bh1r[0:1, 0:P], ones[0:1, :],
                         start=True, stop=True, skip_group_check=True)

        CWX = BC // NCH
        chs = [(ch, slice(ch * CWX, (ch + 1) * CWX)) for ch in range(NCH)]

        # engine spread: (chain, layer 1/2/3) -> relu engine.
        # GPSIMD/Pool cannot access PSUM, so PSUM-reading ops (relu/exp/add)
        # go to ACT+DVE; Pool gets the SBUF-only tail multiply.
        RELU_ENG = {(0, 1): "act", (0, 2): "dve", (0, 3): "act",
                    (1, 1): "dve", (1, 2): "act", (1, 3): "dve"}

        def relu_op(eng, out_ap, in_ap):
            if eng == "act":
                nc.scalar.activation(out_ap, in_ap,
                                     mybir.ActivationFunctionType.Relu)
            elif eng == "dve":
                nc.vector.tensor_scalar_max(out_ap, in_ap, 0.0)
            else:
                nc.gpsimd.tensor_scalar_max(out_ap, in_ap, 0.0)



        streams = [[] for _ in range(NCH)]

        def ph(ch, fn):
            streams[ch].append(fn)

        def mk(fn, *args):
            return lambda a=args: fn(*a)

        k1_off = prep["k1_off"]
        g2_off, g3_off, to_off = prep["g2_off"], prep["g3_off"], prep["to_off"]

        def emit_catchup(ch, hs, idx):
            for Bb in catch_at.get(idx, []):
                nc.tensor.matmul(pre1[Bb % 2][:, hs],
                                 w0b[:, Bb * P:(Bb + 1) * P], xT[:, hs],
                                 start=True, stop=True, skip_group_check=True)

        def emit_k1(ch, hs, idx):
            g = idx - 1
            for Bb in cov[g]:
                off = k1_off[(idx, Bb)]
                nc.tensor.matmul(pre1[Bb % 2][:, hs],
                                 w0k1[0:1, off * P:(off + 1) * P],
                                 xi[(idx - 1) % 2][0:1, hs],
                                 start=False, stop=True, skip_group_check=True)

        def emit_relu(ch, hs, idx, lyr):
            g = idx - 1
            src, dst = {1: (pre1, a1), 2: (s2, a2), 3: (s3, a3)}[lyr]
            for Bb in cov[g]:
                relu_op(RELU_ENG[(ch % 2, lyr)], dst[Bb][:, hs],
                        src[Bb % 2][:, hs])

        def emit_grp(ch, hs, idx, lyr):
            """Add the newly final group g to cover-block accumulators."""
            g = idx - 1
            wg, goff, sb, a_in = {2: (wg2, g2_off, s2, a1),
                                  3: (wg3, g3_off, s3, a2)}[lyr]
            for Bb in cov[g]:
                for (c, lo, hi) in srcs[g]:
                    off = goff[(idx, Bb, c)]
                    nc.tensor.matmul(sb[Bb % 2][:, hs],
                                     wg[:, off * P:(off + 1) * P],
                                     a_in[c][:, hs],
                                     start=False, stop=True,
                                     skip_group_check=True)

        def emit_hid_catchup(ch, hs, idx, lyr):
            """Accumulate bias + finalized prefix for soon-to-be-cover blocks."""
            g = idx - 1
            U = ghi[g]
            wh, sb, a_in, bias = {2: (wh0, s2, a1, bh0r),
                                  3: (wh1, s3, a2, bh1r)}[lyr]
            for Bb in catch_at.get(idx, []):
                dst = sb[Bb % 2][:, hs]
                cfull, rem = U // P, U % P
                terms = [(bias[0:1, Bb * P:(Bb + 1) * P], ones[0:1, hs])]
                terms += [(wh[c][:, Bb * P:(Bb + 1) * P], a_in[c][:, hs])
                          for c in range(cfull)]
                if rem:
                    terms.append((wh[cfull][0:rem, Bb * P:(Bb + 1) * P],
                                  a_in[cfull][0:rem, hs]))
                for j, (lh, rh) in enumerate(terms):
                    nc.tensor.matmul(dst, lh, rh,
                                     start=(j == 0),
                                     stop=(j == len(terms) - 1),
                                     skip_group_check=True)

        def emit_extract(ch, hs, idx):
            # ex row 0 <- ls_idx, ex row 32 <- mu_idx (both %32-aligned).
            # Term 0 selects the theta_sb state row (has groups <= idx-2);
            # wpm terms add the current group's correction from a3.
            cor = srcs[idx - 1] if idx >= 1 else []
            for row, col0 in ((0, idx), (D, D + idx)):
                seq = [(ohx[:, col0:col0 + 1], theta_sb[:, hs])]
                for (c, lo, hi) in cor:
                    off = to_off[(idx, c)]
                    wc = 2 * off if row == 0 else 2 * off + 1
                    seq.append((wpm[:, wc:wc + 1], a3[c][:, hs]))
                for j, (lh, rh) in enumerate(seq):
                    nc.tensor.matmul(ex[row:row + 1, hs], lh, rh,
                                     start=(j == 0), stop=(j == len(seq) - 1),
                                     skip_group_check=True)

        def emit_touch(ch, hs, idx):
            g = idx - 1
            for (c, lo, hi) in srcs[g]:
                off = to_off[(idx, c)]
                nc.tensor.matmul(theta[:, hs],
                                 wto[:, off * 2 * D:(off + 1) * 2 * D],
                                 a3[c][:, hs],
                                 start=False, stop=True,
                                 skip_group_check=True)

        def emit_thsb(ch, hs, idx):
            nc.vector.tensor_scalar_add(theta_sb[:, hs], theta[:, hs], 0.0)

        def emit_exp(ch, hs, idx):
            if ch == 0:
                fetch_z(idx + 3)
            nc.scalar.activation(es[0:1, hs], ex[0:1, hs],
                                 mybir.ActivationFunctionType.Exp)

        def emit_mul(ch, hs, idx):
            nc.vector.tensor_tensor(t2[0:1, hs], es[0:1, hs],
                                    zrow[idx][0:1, hs], mybir.AluOpType.mult)

        def emit_add(ch, hs, idx):
            nc.vector.tensor_tensor(xi[idx % 2][0:1, hs], t2[0:1, hs],
                                    ex[D:D + 1, hs], mybir.AluOpType.add)
            # lazy xT row fill for later catch-ups + final output (SP idle)
            nc.sync.dma_start(xT[idx:idx + 1, hs], xi[idx % 2][0:1, hs])

        for ch, hs in chs:
            for idx in range(STOP):
                if idx >= 1:
                    ph(ch, mk(emit_catchup, ch, hs, idx))
                    ph(ch, mk(emit_k1, ch, hs, idx))
                    ph(ch, mk(emit_relu, ch, hs, idx, 1))
                    ph(ch, mk(emit_grp, ch, hs, idx, 2))
                    ph(ch, mk(emit_hid_catchup, ch, hs, idx, 2))
                    ph(ch, mk(emit_relu, ch, hs, idx, 2))
                    ph(ch, mk(emit_grp, ch, hs, idx, 3))
                    ph(ch, mk(emit_hid_catchup, ch, hs, idx, 3))
                    ph(ch, mk(emit_relu, ch, hs, idx, 3))
                    ph(ch, mk(emit_extract, ch, hs, idx))
                    ph(ch, mk(emit_touch, ch, hs, idx))
                    ph(ch, mk(emit_thsb, ch, hs, idx))
                else:
                    for _ in range(9):
                        ph(ch, lambda: None)
                    ph(ch, mk(emit_extract, ch, hs, idx))
                    for _ in range(2):
                        ph(ch, lambda: None)
                ph(ch, mk(emit_exp, ch, hs, idx))
                ph(ch, mk(emit_mul, ch, hs, idx))
                ph(ch, mk(emit_add, ch, hs, idx))

        # interleaved emission with skew
        total = len(streams[0])
        for i in range(total + SKEW * NCH):
            for ch in range(NCH):
                j = i - ch * SKEW
                if 0 <= j < total:
                    streams[ch][j]()

        nc.scalar.activation(outf[:, :], xT[0:D, :],
                             mybir.ActivationFunctionType.Copy)
        nc.sync.dma_start(d_out, outf[:, :])

    nc.compile()
    return nc


_CACHE = {}


def _get_program(prep):
    if "nc" not in _CACHE:
        _CACHE["nc"] = _build(prep)
    return _CACHE["nc"]


def _in_maps(inputs, prep):
    import ml_dtypes
    bf16 = ml_dtypes.bfloat16
    z = np.asarray(inputs["z"], dtype=np.float32)
    maps = []
    for c in range(NCORES):
        zs = z[c * BC:(c + 1) * BC, :]                 # [512, 32]
        maps.append({
            "w0b": prep["w0b"].astype(bf16),
            "wh0t": prep["wh0T"].astype(bf16),
            "wh1t": prep["wh1T"].astype(bf16),
            "w0k1": prep["w0k1"].astype(bf16),
            "wg2": prep["wg2"].astype(bf16),
            "wg3": prep["wg3"].astype(bf16),
            "wto": prep["wto"].astype(bf16),
            "wpm": prep["wpm"].astype(bf16),
            "ohx": prep["ohx"].astype(bf16),
            "bh0r": prep["bh0r"].astype(bf16),
            "bh1r": prep["bh1r"].astype(bf16),
            "boutr": prep["boutr"].astype(bf16),
            "zb": np.ascontiguousarray(zs.T),          # [32, 512] f32
        })
    return maps


def _prep_from_inputs(inputs):
    return _host_prep(np.asarray(inputs["W0"], np.float32),
                      np.asarray(inputs["b0"], np.float32),
                      np.asarray(inputs["Wh"], np.float32),
                      np.asarray(inputs["bh"], np.float32),
                      np.asarray(inputs["Wout"], np.float32),
                      np.asarray(inputs["bout"], np.float32))


def _run(inputs, trace=False):
    prep = _prep_from_inputs(inputs)
    nc = _get_program(prep)
    maps = _in_maps(inputs, prep)
    res = run_bass_kernel_spmd(nc, maps, core_ids=list(range(NCORES)),
                               trace=trace)
    out = np.empty((B, D), dtype=np.float32)
    for c in range(NCORES):
        out[c * BC:(c + 1) * BC, :] = res.results[c]["out"].T
    return out, res


def kernel(**inputs):
    out, _ = _run(inputs, trace=False)
    return out


# revision 38
# speedup vs baseline: 1.0127x; 1.0127x over previous
"""Incremental MADE autoregressive sampler on 8 TRN2 NeuronCores.

v2: ALL layer accumulators are persistent PSUM banks updated incrementally.

With hidden units degree-sorted, activations are append-only across the 32
autoregressive steps: once x_0..x_g are set, every hidden unit of degree <= g
is final. Exploit this at every layer:

- pre1 (L1 pre-activations): ping-pong PSUM banks per 128-block; one K=1
  rank-1 update per step (new coordinate), plus a single K=33 catch-up matmul
  from xT (with a ones-row folding in the bias) when a block first becomes
  the active cover.
- S2/S3 (L2/L3 pre-activations): ping-pong PSUM banks per cover block. Each
  step adds ONLY the newly finalized ~33-unit degree group (K~33 matmul).
  When a block is about to become cover, a catch-up chain (bias + full
  finalized chunks) accumulates the older prefix once. No per-step prefix
  recompute -> Tensor queue no longer head-of-line-blocks the serial chain.
- theta [64, B]: single persistent PSUM accumulator; per-step "touch" adds
  the new group's contribution to all output rows (K~33, M=64). The tail
  reads rows idx (mu) and 32+idx (log_std) straight out of PSUM.
- Tail: es=exp(theta_ls) (ACT) -> t2=es*z -> x_idx=t2+theta_mu written
  DIRECTLY into the xT row in SBUF (no DMA scatter; k1/catch-up read xT).

Batch: data-parallel over 8 cores (512 rows/core); each core runs two
256-wide column chains, phase-interleaved with a skew so the two serial
dependency chains overlap on different engines. Relus/tails are spread
across ACT/DVE/Pool(gpsimd) so no single elementwise engine saturates.
"""

import os
import sys
import numpy as np

for _p in ("/opt/trn_rl_repo", "/opt/pypackages"):
    if _p not in sys.path:
        sys.path.insert(0, _p)

import concourse.bass as bass
import concourse.tile as tile
from concourse import bacc
from concourse import mybir
from concourse.bass_utils import run_bass_kernel_spmd

D, H, L, B = 32, 1024, 2, 4096
NCORES = 8
BC = B // NCORES          # 512 batch rows per core
P = 128                   # partitions
NB = H // P               # 8 hidden blocks
F32 = mybir.dt.float32
MMDT = mybir.dt.bfloat16

STOP = int(os.environ.get("MADE_STOP", "32"))
NCH = int(os.environ.get("MADE_CHAINS", "2"))
SKEW = int(os.environ.get("MADE_SKEW", "7"))


def _schedule():
    """Static per-step schedule from the degree structure."""
    d_hid = np.arange(H) % (D - 1)
    perm = np.argsort(d_hid, kind="stable")
    ds = d_hid[perm]
    glo = [int(np.sum(ds < g)) for g in range(D - 1)]
    ghi = [int(np.sum(ds <= g)) for g in range(D - 1)]
    cov = {g: list(range(glo[g] // P, (ghi[g] - 1) // P + 1))
           for g in range(D - 1)}
    # src_slices(g): (block c, row lo, row hi) covering units [glo, ghi)
    srcs = {}
    for g in range(D - 1):
        sl = []
        for c in cov[g]:
            lo = max(glo[g], c * P) - c * P
            hi = min(ghi[g], (c + 1) * P) - c * P
            sl.append((c, lo, hi))
        srcs[g] = sl
    # first step idx at which block Bb is in cover
    first = {}
    for g in range(D - 1):
        for Bb in cov[g]:
            first.setdefault(Bb, g + 1)
    return perm, ds, glo, ghi, cov, srcs, first


def _host_prep(W0, b0, Wh, bh, Wout, bout):
    perm, ds, glo, ghi, cov, srcs, first = _schedule()
    d_in = np.arange(D)
    d_out = np.arange(D) - 1
    m0 = (ds[:, None] >= d_in[None, :]).astype(np.float32)        # [H, D]
    mh = (ds[:, None] >= ds[None, :]).astype(np.float32)          # [H, H]
    mo = (d_out[:, None] >= ds[None, :]).astype(np.float32)       # [D, H]
    mo = np.concatenate([mo, mo], axis=0)                         # [2D, H]

    W0p = m0 * W0[perm, :]                    # [H, D] (out, in)
    Wh0p = mh * Wh[0][perm][:, perm]          # [H, H] (out, in)
    Wh1p = mh * Wh[1][perm][:, perm]
    Wop = mo * Wout[:, perm]                  # [2D, H]

    w0b = np.concatenate([W0p.T, b0[perm][None, :]], axis=0)      # [D+1, H]
    wh0T = Wh0p.T.reshape(NB, P, H).copy()                        # [c][128, H]
    wh1T = Wh1p.T.reshape(NB, P, H).copy()

    # k1 packed columns: per (idx, Bb in cov(idx-1)): W0p[block Bb, idx-1]
    k1_off, k1_list = {}, []
    for idx in range(1, D):
        for Bb in cov[idx - 1]:
            k1_off[(idx, Bb)] = len(k1_list)
            k1_list.append(W0p[Bb * P:(Bb + 1) * P, idx - 1])
    w0k1 = np.concatenate(k1_list).reshape(1, -1)                 # [1, n*128]

    # group matmul lhsT slices, zero-masked outside group rows, base-0 K=128:
    # per (idx, Bb dst, c src): whT[c][:, Bb block] with rows outside
    # [lo, hi) zeroed -> [128, 128]
    def pack_grp(whT):
        off, lst = {}, []
        for idx in range(1, D):
            g = idx - 1
            for Bb in cov[g]:
                for (c, lo, hi) in srcs[g]:
                    blk = whT[c][:, Bb * P:(Bb + 1) * P].copy()
                    blk[:lo, :] = 0.0
                    blk[hi:, :] = 0.0
                    off[(idx, Bb, c)] = len(lst)
                    lst.append(blk)
        return off, np.concatenate(lst, axis=1)                   # [128, n*128]

    g2_off, wg2 = pack_grp(wh0T)
    g3_off, wg3 = pack_grp(wh1T)

    # touch lhsT slices: per (idx, c src): Wop[:, block c].T rows-masked
    to_off, to_list = {}, []
    for idx in range(1, D):
        g = idx - 1
        for (c, lo, hi) in srcs[g]:
            blk = Wop[:, c * P:(c + 1) * P].T.copy()              # [128, 2D]
            blk[:lo, :] = 0.0
            blk[hi:, :] = 0.0
            to_off[(idx, c)] = len(to_list)
            to_list.append(blk)
    wto = np.concatenate(to_list, axis=1)                         # [128, n*2D]

    # extract lhsT slices, M=33 (ls -> out row 0, mu -> out row 32):
    # wpm[(idx, c)]: [128, 33] with col 0 = masked ls col, col 32 = mu col
    M3 = D + 1
    wpm = np.zeros((P, len(to_list) * M3), dtype=np.float32)
    for (idx, c), off in to_off.items():
        wpm[:, M3 * off] = to_list[off][:, D + idx]               # ls col
        wpm[:, M3 * off + D] = to_list[off][:, idx]               # mu col

    # one-hot theta_sb selectors, M=33 per idx: col 0 = e_{D+idx}, col 32 = e_idx
    ohx = np.zeros((2 * D, D * M3), dtype=np.float32)
    for idx in range(D):
        ohx[D + idx, M3 * idx] = 1.0
        ohx[idx, M3 * idx + D] = 1.0

    return dict(w0b=w0b, wh0T=wh0T, wh1T=wh1T,
                w0k1=w0k1, wg2=wg2, wg3=wg3, wto=wto, wpm=wpm, ohx=ohx,
                k1_off=k1_off, g2_off=g2_off, g3_off=g3_off, to_off=to_off,
                n_k1=len(k1_list), n_g2=len(g2_off), n_g3=len(g3_off),
                n_to=len(to_list),
                bh0r=bh[0][perm][None, :], bh1r=bh[1][perm][None, :],
                boutr=bout[None, :],
                glo=glo, ghi=ghi, cov=cov, srcs=srcs, first=first, ds=ds)


def _build(prep):
    nc = bacc.Bacc("TRN2", target_bir_lowering=False, debug=False,
                   num_devices=NCORES)

    def din(name, shape, dt=MMDT):
        return nc.dram_tensor(name, list(shape), dt, kind="ExternalInput").ap()

    d_w0b = din("w0b", (D + 1, H))
    d_wh0 = din("wh0t", (NB, P, H))
    d_wh1 = din("wh1t", (NB, P, H))
    d_w0k1 = din("w0k1", (1, prep["n_k1"] * P))
    d_wg2 = din("wg2", (P, prep["n_g2"] * P))
    d_wg3 = din("wg3", (P, prep["n_g3"] * P))
    d_wto = din("wto", (P, prep["n_to"] * 2 * D))
    d_wpm = din("wpm", (P, prep["n_to"] * (D + 1)))
    d_ohx = din("ohx", (2 * D, D * (D + 1)))
    d_bh0 = din("bh0r", (1, H))
    d_bh1 = din("bh1r", (1, H))
    d_bo = din("boutr", (1, 2 * D))
    d_z = din("zb", (D, BC), F32)
    d_out = nc.dram_tensor("out", [D, BC], F32, kind="ExternalOutput").ap()

    cov, srcs, first = prep["cov"], prep["srcs"], prep["first"]
    ghi, dsl = prep["ghi"], prep["ds"]
    # pre1 catch-up for block Bb is emitted during step first[Bb]-1
    catch_at = {}
    for Bb, f in first.items():
        if Bb >= 1:
            catch_at.setdefault(f - 1, []).append(Bb)

    # S2/S3 catch-up terms, spread over steps f-3..f-1 by data availability
    # (chunk c of the prefix is final once its last unit's group is done).
    # Terms of one (lyr, Bb) accumulation group stay in order; start/stop
    # flags mark the PSUM group boundaries.
    hc_sched = {}
    for lyr in (2, 3):
        for Bb in range(1, NB):
            f = first[Bb]
            U = ghi[f - 2]
            cfull, rem = U // P, U % P
            terms = [("bias", Bb)]
            terms += [("chunk", Bb, c) for c in range(cfull)]
            if rem:
                terms.append(("part", Bb, cfull, rem))
            n = len(terms)
            for j, t in enumerate(terms):
                if t[0] == "bias":
                    e = f - 3
                elif t[0] == "chunk":
                    e = max(f - 3, int(dsl[(t[2] + 1) * P - 1]) + 1)
                else:
                    e = f - 1
                e = min(max(e, 1), f - 1)
                hc_sched.setdefault((lyr, e), []).append((t, j == 0, j == n - 1))

    from contextlib import ExitStack
    with tile.TileContext(nc) as tc, ExitStack() as ctx:
        cp = ctx.enter_context(tc.tile_pool(name="const", bufs=1))
        pp = ctx.enter_context(tc.tile_pool(name="pers", bufs=1, space="PSUM"))

        w0b = cp.tile([D + 1, H], MMDT, tag="w0b")
        wh0 = [cp.tile([P, H], MMDT, tag=f"wh0_{c}", name=f"wh0_{c}")
               for c in range(NB)]
        wh1 = [cp.tile([P, H], MMDT, tag=f"wh1_{c}", name=f"wh1_{c}")
               for c in range(NB)]
        w0k1 = cp.tile([1, prep["n_k1"] * P], MMDT, tag="w0k1")
        wg2 = cp.tile([P, prep["n_g2"] * P], MMDT, tag="wg2")
        wg3 = cp.tile([P, prep["n_g3"] * P], MMDT, tag="wg3")
        wto = cp.tile([P, prep["n_to"] * 2 * D], MMDT, tag="wto")
        bh0r = cp.tile([1, H], MMDT, tag="bh0r")
        bh1r = cp.tile([1, H], MMDT, tag="bh1r")
        bor = cp.tile([1, 2 * D], MMDT, tag="bor")
        wpm = cp.tile([P, prep["n_to"] * (D + 1)], MMDT, tag="wpm")
        ohx = cp.tile([2 * D, D * (D + 1)], MMDT, tag="ohx")
        theta_sb = cp.tile([2 * D, BC], MMDT, tag="theta_sb")
        ones = cp.tile([1, BC], MMDT, tag="ones")
        xT = cp.tile([D + 1, BC], MMDT, tag="xT")
        a1 = [cp.tile([P, BC], MMDT, tag=f"a1_{r}", name=f"a1_{r}") for r in range(NB)]
        a2 = [cp.tile([P, BC], MMDT, tag=f"a2_{r}", name=f"a2_{r}") for r in range(NB)]
        a3 = [cp.tile([P, BC], MMDT, tag=f"a3_{r}", name=f"a3_{r}") for r in range(NB)]
        es = cp.tile([1, BC], F32, tag="es")
        t2 = cp.tile([1, BC], F32, tag="t2")
        xi = [cp.tile([1, BC], MMDT, tag=f"xi{p}", name=f"xi{p}")
              for p in range(2)]
        outf = cp.tile([D, BC], F32, tag="outf")

        # persistent PSUM: pre1/S2/S3 ping-pong banks + theta accumulator +
        # extract tile (ls at row 0, mu at row 32: both %32-aligned)
        pre1 = [pp.tile([P, BC], F32, tag=f"pre1_{s}", name=f"pre1_{s}")
                for s in range(2)]
        s2 = [pp.tile([P, BC], F32, tag=f"s2_{s}", name=f"s2_{s}")
              for s in range(2)]
        s3 = [pp.tile([P, BC], F32, tag=f"s3_{s}", name=f"s3_{s}")
              for s in range(2)]
        theta = pp.tile([2 * D, BC], F32, tag="theta")
        ex = pp.tile([D + 1, BC], F32, tag="ex")

        # input DMAs, ordered by first use; big packed tensors are issued in
        # quarters (columns are step-ordered) so early steps aren't stalled
        # behind the whole tensor.
        def quarters(tl, dr, ncols):
            out = []
            bnd = [0] + [ncols * k // 4 for k in (1, 2, 3)] + [ncols]
            for a, b in zip(bnd, bnd[1:]):
                out.append((tl, dr, a, b))
            return out

        qg2 = quarters(wg2, d_wg2, prep["n_g2"] * P)
        qg3 = quarters(wg3, d_wg3, prep["n_g3"] * P)
        qto = quarters(wto, d_wto, prep["n_to"] * 2 * D)
        qpm = quarters(wpm, d_wpm, prep["n_to"] * (D + 1))

        def dq(q):
            tl, dr, a, b = q
            nc.sync.dma_start(tl[:, a:b], dr[:, a:b])

        nc.sync.dma_start(bor[:], d_bo)
        nc.sync.dma_start(ohx[:], d_ohx)
        nc.sync.dma_start(w0b[:], d_w0b)
        nc.sync.dma_start(w0k1[:], d_w0k1)
        nc.sync.dma_start(bh0r[:], d_bh0)
        nc.sync.dma_start(bh1r[:], d_bh1)
        for k in (0, 1):
            dq(qpm[k]); dq(qg2[k]); dq(qg3[k]); dq(qto[k])
        for c in range(4):
            nc.sync.dma_start(wh0[c][:], d_wh0[c, :, :])
            nc.sync.dma_start(wh1[c][:], d_wh1[c, :, :])
        for k in (2, 3):
            dq(qpm[k]); dq(qg2[k]); dq(qg3[k]); dq(qto[k])
        for c in range(4, NB):
            nc.sync.dma_start(wh0[c][:], d_wh0[c, :, :])
            nc.sync.dma_start(wh1[c][:], d_wh1[c, :, :])

        zrow = {}

        def fetch_z(i):
            if i < STOP and i not in zrow:
                zr_t = cp.tile([1, BC], F32, tag="zrow", bufs=4, name=f"zr{i}")
                zrow[i] = zr_t
                nc.sync.dma_start(zr_t[:], d_z[i:i + 1, :])

        for i in range(3):
            fetch_z(i)

        nc.vector.memset(xT[:], 0.0)
        nc.vector.memset(xT[D:D + 1, :], 1.0)
        nc.vector.memset(ones[:], 1.0)

        # theta := bout broadcast (rank-1); block-0 accumulators: bias+coords
        nc.tensor.matmul(theta[:, :], bor[0:1, :], ones[0:1, :],
                         start=True, stop=True, skip_group_check=True)
        nc.vector.tensor_scalar_add(theta_sb[:, :], theta[:, :], 0.0)
        nc.tensor.matmul(pre1[0], w0b[:, 0:P], xT[:, :],
                         start=True, stop=True, skip_group_check=True)
        nc.tensor.matmul(s2[0], bh0r[0:1, 0:P], ones[0:1, :],
                         start=True, stop=True, skip_group_check=True)
        nc.tensor.matmul(s3[0], bh1r[0:1, 0:P], ones[0:1, :],
                         start=True, stop=True, skip_group_check=True)

        CWX = BC // NCH
        chs = [(ch, slice(ch * CWX, (ch + 1) * CWX)) for ch in range(NCH)]

        # engine spread: (chain, layer 1/2/3) -> relu engine.
        # GPSIMD/Pool cannot access PSUM, so PSUM-reading ops (relu/exp/add)
        # go to ACT+DVE; Pool gets the SBUF-only tail multiply.
        RELU_ENG = {(0, 1): "act", (0, 2): "dve", (0, 3): "act",
                    (1, 1): "dve", (1, 2): "act", (1, 3): "dve"}

        def relu_op(eng, out_ap, in_ap):
            if eng == "act":
                nc.scalar.activation(out_ap, in_ap,
                                     mybir.ActivationFunctionType.Relu)
            elif eng == "dve":
                nc.vector.tensor_scalar_max(out_ap, in_ap, 0.0)
            else:
                nc.gpsimd.tensor_scalar_max(out_ap, in_ap, 0.0)



        streams = [[] for _ in range(NCH)]

        def ph(ch, fn):
            streams[ch].append(fn)

        def mk(fn, *args):
            return lambda a=args: fn(*a)

        k1_off = prep["k1_off"]
        g2_off, g3_off, to_off = prep["g2_off"], prep["g3_off"], prep["to_off"]

        def emit_catchup(ch, hs, idx):
            for Bb in catch_at.get(idx, []):
                nc.tensor.matmul(pre1[Bb % 2][:, hs],
                                 w0b[:, Bb * P:(Bb + 1) * P], xT[:, hs],
                                 start=True, stop=True, skip_group_check=True)

        def emit_k1(ch, hs, idx):
            g = idx - 1
            for Bb in cov[g]:
                off = k1_off[(idx, Bb)]
                nc.tensor.matmul(pre1[Bb % 2][:, hs],
                                 w0k1[0:1, off * P:(off + 1) * P],
                                 xi[(idx - 1) % 2][0:1, hs],
                                 start=False, stop=True, skip_group_check=True)

        def emit_relu(ch, hs, idx, lyr):
            g = idx - 1
            src, dst = {1: (pre1, a1), 2: (s2, a2), 3: (s3, a3)}[lyr]
            for Bb in cov[g]:
                relu_op(RELU_ENG[(ch % 2, lyr)], dst[Bb][:, hs],
                        src[Bb % 2][:, hs])

        def emit_grp(ch, hs, idx, lyr):
            """Add the newly final group g to cover-block accumulators."""
            g = idx - 1
            wg, goff, sb, a_in = {2: (wg2, g2_off, s2, a1),
                                  3: (wg3, g3_off, s3, a2)}[lyr]
            for Bb in cov[g]:
                for (c, lo, hi) in srcs[g]:
                    off = goff[(idx, Bb, c)]
                    nc.tensor.matmul(sb[Bb % 2][:, hs],
                                     wg[:, off * P:(off + 1) * P],
                                     a_in[c][:, hs],
                                     start=False, stop=True,
                                     skip_group_check=True)

        def emit_hid_catchup(ch, hs, idx, lyr):
            """Accumulate bias + finalized prefix for soon-to-be-cover blocks."""
            wh, sb, a_in, bias = {2: (wh0, s2, a1, bh0r),
                                  3: (wh1, s3, a2, bh1r)}[lyr]
            for (t, is_start, is_stop) in hc_sched.get((lyr, idx), []):
                Bb = t[1]
                dst = sb[Bb % 2][:, hs]
                if t[0] == "bias":
                    lh, rh = bias[0:1, Bb * P:(Bb + 1) * P], ones[0:1, hs]
                elif t[0] == "chunk":
                    c = t[2]
                    lh, rh = wh[c][:, Bb * P:(Bb + 1) * P], a_in[c][:, hs]
                else:
                    c, rem = t[2], t[3]
                    lh, rh = (wh[c][0:rem, Bb * P:(Bb + 1) * P],
                              a_in[c][0:rem, hs])
                nc.tensor.matmul(dst, lh, rh, start=is_start, stop=is_stop,
                                 skip_group_check=True)

        def emit_extract(ch, hs, idx):
            # One M=33 chain: ls_idx -> ex row 0, mu_idx -> ex row 32 (both
            # %32-aligned). Term 0 selects the theta_sb state rows (groups
            # <= idx-2); wpm terms add the current group's correction.
            M3 = D + 1
            cor = srcs[idx - 1] if idx >= 1 else []
            seq = [(ohx[:, M3 * idx:M3 * (idx + 1)], theta_sb[:, hs])]
            for (c, lo, hi) in cor:
                off = to_off[(idx, c)]
                seq.append((wpm[:, M3 * off:M3 * (off + 1)], a3[c][:, hs]))
            for j, (lh, rh) in enumerate(seq):
                nc.tensor.matmul(ex[:, hs], lh, rh,
                                 start=(j == 0), stop=(j == len(seq) - 1),
                                 skip_group_check=True)

        def emit_touch(ch, hs, idx):
            g = idx - 1
            for (c, lo, hi) in srcs[g]:
                off = to_off[(idx, c)]
                nc.tensor.matmul(theta[:, hs],
                                 wto[:, off * 2 * D:(off + 1) * 2 * D],
                                 a3[c][:, hs],
                                 start=False, stop=True,
                                 skip_group_check=True)

        def emit_thsb(ch, hs, idx):
            nc.vector.tensor_scalar_add(theta_sb[:, hs], theta[:, hs], 0.0)

        def emit_exp(ch, hs, idx):
            if ch == 0:
                fetch_z(idx + 3)
            nc.scalar.activation(es[0:1, hs], ex[0:1, hs],
                                 mybir.ActivationFunctionType.Exp)

        def emit_mul(ch, hs, idx):
            nc.vector.tensor_tensor(t2[0:1, hs], es[0:1, hs],
                                    zrow[idx][0:1, hs], mybir.AluOpType.mult)

        def emit_add(ch, hs, idx):
            nc.vector.tensor_tensor(xi[idx % 2][0:1, hs], t2[0:1, hs],
                                    ex[D:D + 1, hs], mybir.AluOpType.add)
            # lazy xT row fill for later catch-ups + final output (SP idle)
            nc.sync.dma_start(xT[idx:idx + 1, hs], xi[idx % 2][0:1, hs])

        for ch, hs in chs:
            for idx in range(STOP):
                if idx >= 1:
                    ph(ch, mk(emit_catchup, ch, hs, idx))
                    ph(ch, mk(emit_k1, ch, hs, idx))
                    ph(ch, mk(emit_relu, ch, hs, idx, 1))
                    ph(ch, mk(emit_grp, ch, hs, idx, 2))
                    ph(ch, mk(emit_hid_catchup, ch, hs, idx, 2))
                    ph(ch, mk(emit_relu, ch, hs, idx, 2))
                    ph(ch, mk(emit_grp, ch, hs, idx, 3))
                    ph(ch, mk(emit_hid_catchup, ch, hs, idx, 3))
                    ph(ch, mk(emit_relu, ch, hs, idx, 3))
                    ph(ch, mk(emit_extract, ch, hs, idx))
                    ph(ch, mk(emit_touch, ch, hs, idx))
                    ph(ch, mk(emit_thsb, ch, hs, idx))
                else:
                    for _ in range(9):
                        ph(ch, lambda: None)
                    ph(ch, mk(emit_extract, ch, hs, idx))
                    for _ in range(2):
                        ph(ch, lambda: None)
                ph(ch, mk(emit_exp, ch, hs, idx))
                ph(ch, mk(emit_mul, ch, hs, idx))
                ph(ch, mk(emit_add, ch, hs, idx))

        # interleaved emission with skew
        total = len(streams[0])
        for i in range(total + SKEW * NCH):
            for ch in range(NCH):
                j = i - ch * SKEW
                if 0 <= j < total:
                    streams[ch][j]()

        nc.scalar.activation(outf[:, :], xT[0:D, :],
                             mybir.ActivationFunctionType.Copy)
        nc.sync.dma_start(d_out, outf[:, :])

    nc.compile()
    return nc


_CACHE = {}


def _get_program(prep):
    if "nc" not in _CACHE:
        _CACHE["nc"] = _build(prep)
    return _CACHE["nc"]


def _in_maps(inputs, prep):
    import ml_dtypes
    bf16 = ml_dtypes.bfloat16
    z = np.asarray(inputs["z"], dtype=np.float32)
    maps = []
    for c in range(NCORES):
        zs = z[c * BC:(c + 1) * BC, :]                 # [512, 32]
        maps.append({
            "w0b": prep["w0b"].astype(bf16),
            "wh0t": prep["wh0T"].astype(bf16),
            "wh1t": prep["wh1T"].astype(bf16),
            "w0k1": prep["w0k1"].astype(bf16),
            "wg2": prep["wg2"].astype(bf16),
            "wg3": prep["wg3"].astype(bf16),
            "wto": prep["wto"].astype(bf16),
            "wpm": prep["wpm"].astype(bf16),
            "ohx": prep["ohx"].astype(bf16),
            "bh0r": prep["bh0r"].astype(bf16),
            "bh1r": prep["bh1r"].astype(bf16),
            "boutr": prep["boutr"].astype(bf16),
            "zb": np.ascontiguousarray(zs.T),          # [32, 512] f32
        })
    return maps


def _prep_from_inputs(inputs):
    return _host_prep(np.asarray(inputs["W0"], np.float32),
                      np.asarray(inputs["b0"], np.float32),
                      np.asarray(inputs["Wh"], np.float32),
                      np.asarray(inputs["bh"], np.float32),
                      np.asarray(inputs["Wout"], np.float32),
                      np.asarray(inputs["bout"], np.float32))


def _run(inputs, trace=False):
    prep = _prep_from_inputs(inputs)
    nc = _get_program(prep)
    maps = _in_maps(inputs, prep)
    res = run_bass_kernel_spmd(nc, maps, core_ids=list(range(NCORES)),
                               trace=trace)
    out = np.empty((B, D), dtype=np.float32)
    for c in range(NCORES):
        out[c * BC:(c + 1) * BC, :] = res.results[c]["out"].T
    return out, res


def kernel(**inputs):
    out, _ = _run(inputs, trace=False)
    return out
